# revision 11
# baseline (speedup 1.0000x reference)
"""Trainium2 Bass kernel for nn_DLI_loss_full.

Key algebraic fact: logits[b,j,k] = hw[b,j] + xw[b,k] and the loss is
sum(lse - tgt) over valid groups, so the hw[b,j] term (the whole LSTM
path) cancels exactly:

    per_group[b,j] = log(sum_{k=j+1}^{len_b-1} exp(xw[b,k])) - xw[b,j+1]
    loss = sum(per_group) / sum_b(len_b - 1)

with xw = encoder_output @ w_fc[HID:].  The kernel streams
encoder_output once (memory-bound, ~351 B/ns on the scalar HWDGE queue
with all 16 DMA engines saturated) and computes the per-timestep dot
products with all three compute engines balanced against that stream:

  - scalar (ACT) casts each landed fp32 piece to bf16 and does two
    pieces' 256-wide reductions via Copy+accum_out,
  - vector (DVE) multiplies in the 2x 16-bit perf mode and reduces the
    remaining pieces (reduction runs 1x, hence the offloads),
  - gpsimd multiplies two pieces in fp32 straight from the landed
    tiles (no cast needed).

All weight replicas / masks ship pre-built from the host in two
constant blobs (one fp32, one bf16) so nothing is replicated or cast
on the critical path; the blobs ride the sync HWDGE queue as one
descriptor per partition to avoid stealing DMA issue slots from the x
stream.  A manual LoadActFuncSet of natural_log_exp_and_others at the
top of the scalar stream serves Exp and Ln with a single hidden table
load.  The valid-k mask folds into xw as an additive -60000 bias so a
single Exp+accum produces both the masked exponentials and the chunk
totals; suffix log-sum-exps then come from one per-chunk scan seeded
through a 128x128 bf16 matmul.
"""

from contextlib import ExitStack

import ml_dtypes
import numpy as np

import concourse.bacc as bacc
import concourse.mybir as mybir
import concourse.tile as tile
from concourse import bass_utils

B, T, D, HID = 128, 384, 256, 256
NCORES = 8
BS = B // NCORES            # 16 batches per core
CH = 8                      # chunks per sequence
L = T // CH                 # 48 timesteps per chunk
P = BS * CH                 # 128 partitions
NP = 8                      # DMA/compute pieces along the free axis
LP = L // NP                # 6 timesteps per piece
F32 = mybir.dt.float32
BF16 = mybir.dt.bfloat16
EPS = 1e-30                 # keeps ln() finite on fully-masked tails
MASK_NEG = -60000.0         # exp() underflows to exactly 0
ATL_LN_EXP = 6              # act_info.json index of natural_log_exp_and_others

GP_MULT = (1, 4)            # pieces whose multiply runs on gpsimd (fp32)
ACT_RED = (2, 5)            # pieces whose reduce runs on scalar Copy+accum

_cache = {}


def _build_nc():
    nc = bacc.Bacc(
        "TRN2", target_bir_lowering=False, debug=False, num_devices=NCORES
    )
    x = nc.dram_tensor("x", [BS, T, D], F32, kind="ExternalInput").ap()
    # constant blobs: one descriptor per partition each
    cb = nc.dram_tensor("cb", [P, LP * D + 3 * L], F32, kind="ExternalInput").ap()
    c2 = nc.dram_tensor("c2", [P, LP * D + P], BF16, kind="ExternalInput").ap()
    out = nc.dram_tensor("out", [P, 2], F32, kind="ExternalOutput").ap()

    add = mybir.AluOpType.add
    mult = mybir.AluOpType.mult
    bypass = mybir.AluOpType.bypass
    AX = mybir.AxisListType.X
    ACT = mybir.ActivationFunctionType

    with tile.TileContext(nc) as tc, ExitStack() as ctx, \
            nc.allow_low_precision(reason="bf16 2x-mode dot products; loss tolerance is 2e-2"):
        sp = ctx.enter_context(tc.tile_pool(name="small", bufs=1))
        xp = ctx.enter_context(tc.tile_pool(name="xp", bufs=NP))
        bp = ctx.enter_context(tc.tile_pool(name="bp", bufs=4))
        pp = ctx.enter_context(tc.tile_pool(name="psum", bufs=1, space="PSUM"))

        # one table load serves every Exp/Ln below; runs at stream head
        # while the first x piece is still in flight
        nc.scalar.add_instruction(
            mybir.InstLoadActFuncSet(
                name="manual_atl", act_func_set_id=ATL_LN_EXP, ins=[], outs=[]
            )
        )

        # x-piece loads all on the scalar HWDGE queue: 128 descriptors of
        # 6KB per piece keep all 16 DMA engines saturated
        x_p = x.rearrange("b (c n l) d -> (b c) n (l d)", c=CH, n=NP)
        xts = []
        for i in range(NP):
            xt = xp.tile([P, LP * D], F32, tag="x")
            nc.scalar.dma_start(xt[:], x_p[:, i, :])
            xts.append(xt)

        # constant blobs on the sync HWDGE queue
        c_sb = sp.tile([P, LP * D + 3 * L], F32)
        nc.sync.dma_start(c_sb[:], cb)
        wf = c_sb[:, 0:LP * D]
        w3f = wf.rearrange("p (l d) -> p l d", d=D)
        mf_sb = c_sb[:, LP * D:LP * D + L]
        wm_sb = c_sb[:, LP * D + L:LP * D + 2 * L]
        mb_sb = c_sb[:, LP * D + 2 * L:LP * D + 3 * L]
        c2_sb = sp.tile([P, LP * D + P], BF16)
        nc.sync.dma_start(c2_sb[:], c2)
        wb = c2_sb[:, 0:LP * D]
        w3b = wb.rearrange("p (l d) -> p l d", d=D)
        umb = c2_sb[:, LP * D:LP * D + P]

        res = sp.tile([P, 2], F32)
        # valid-group count is independent of x: do it up front
        nc.vector.tensor_reduce(res[:, 1:2], mf_sb, axis=AX, op=add)

        # xw[p, t] = sum_d x[p, t, d] * w[d]
        xw = sp.tile([P, L], F32)
        ascr = sp.tile([P, D], BF16)
        for i in range(NP):
            if i in GP_MULT:
                x3 = xts[i][:].rearrange("p (l d) -> p l d", d=D)
                nc.gpsimd.tensor_tensor(x3, x3, w3f, mult)
                nc.vector.tensor_reduce(
                    xw[:, i * LP:(i + 1) * LP], x3, axis=AX, op=add
                )
            elif i < NP - 1:
                xb = bp.tile([P, LP * D], BF16, tag="xb")
                nc.scalar.activation(xb[:], xts[i][:], ACT.Copy)
                x3 = xb[:].rearrange("p (l d) -> p l d", d=D)
                nc.vector.tensor_tensor(x3, x3, w3b, mult)
                if i in ACT_RED:
                    for l in range(LP):
                        t = i * LP + l
                        nc.scalar.activation(
                            ascr[:], xb[:, l * D:(l + 1) * D], ACT.Copy,
                            accum_out=xw[:, t:t + 1],
                        )
                else:
                    nc.vector.tensor_reduce(
                        xw[:, i * LP:(i + 1) * LP], x3, axis=AX, op=add
                    )
            else:
                # last piece in three 2-row chips to shorten the serial
                # cast->mult->reduce tail after its DMA lands
                xb = bp.tile([P, LP * D], BF16, tag="xb")
                x3 = xb[:].rearrange("p (l d) -> p l d", d=D)
                for j in range(3):
                    c0, c1 = j * 2, (j + 1) * 2
                    nc.scalar.activation(
                        xb[:, c0 * D:c1 * D], xts[i][:, c0 * D:c1 * D],
                        ACT.Copy,
                    )
                    nc.vector.tensor_tensor(
                        x3[:, c0:c1], x3[:, c0:c1], w3b[:, c0:c1], mult
                    )
                    nc.vector.tensor_reduce(
                        xw[:, i * LP + c0:i * LP + c1],
                        x3[:, c0:c1], axis=AX, op=add,
                    )

        # fold the valid-k mask into xw (masked -> -60000, exp -> 0);
        # wm is 0 there so the loss terms are unaffected
        nc.vector.tensor_add(xw[:], xw[:], mb_sb)
        # masked exponentials and chunk totals in one activation
        em = sp.tile([P, L], F32)
        tot = sp.tile([P, 1], BF16)
        nc.scalar.activation(em[:], xw[:], ACT.Exp, accum_out=tot[:])
        # cross-chunk exclusive suffix totals via 128x128 bf16 matmul
        aps = pp.tile([P, 1], F32, tag="mm")
        nc.tensor.matmul(aps[:], umb, tot[:], start=True, stop=True)
        a_sb = sp.tile([P, 1], F32)
        # + EPS seeds every suffix sum, keeping ln() finite on
        # fully-masked tails
        nc.vector.tensor_scalar_add(a_sb[:], aps[:], EPS)

        # within-chunk suffix sums, seeded with the later-chunk total
        ss = sp.tile([P, L], F32)
        nc.vector.tensor_tensor_scan(
            ss[:][:, ::-1], em[:][:, ::-1], em[:][:, ::-1],
            initial=a_sb[:], op0=add, op1=bypass,
        )
        lt = sp.tile([P, L], F32)
        nc.scalar.activation(lt[:], ss[:], ACT.Ln)

        # loss terms: sum over valid groups of (ln(suffix) - xw)
        diff = sp.tile([P, L], F32)
        nc.vector.tensor_sub(diff[:], lt[:], xw[:])
        nc.vector.scalar_tensor_tensor(
            out=diff[:], in0=diff[:], scalar=1.0, in1=wm_sb,
            op0=bypass, op1=mult, accum_out=res[:, 0:1],
        )
        nc.scalar.dma_start(out, res[:])

    nc.compile()
    return nc


def _host_consts():
    w_idx = np.arange(P)
    um = (
        (w_idx[:, None] // CH == w_idx[None, :] // CH)
        & (w_idx[:, None] % CH > w_idx[None, :] % CH)
    ).astype(np.float32)
    cm = np.ones((P, L), np.float32)
    cm[w_idx % CH == 0, 0] = 0.0
    return um, cm


def _host_blobs(mask, w_fc):
    """Per-core (cb fp32, shared c2 bf16) constant blobs."""
    um, cm = _host_consts()
    wrep = np.tile(w_fc[HID:], LP)[None, :].repeat(P, 0).astype(np.float32)
    c2 = np.concatenate(
        [wrep, um], axis=1
    ).astype(ml_dtypes.bfloat16)
    mfs = mask.astype(np.float32).reshape(NCORES, P, L)
    cbs = []
    for c in range(NCORES):
        mf = mfs[c]
        wm = mf * cm
        mb = (1.0 - mf) * MASK_NEG
        cbs.append(np.ascontiguousarray(
            np.concatenate([wrep, mf, wm, mb], axis=1), np.float32
        ))
    return cbs, np.ascontiguousarray(c2)


def kernel(**inputs) -> np.ndarray:
    enc = np.ascontiguousarray(np.asarray(inputs["encoder_output"], np.float32))
    mask = np.ascontiguousarray(np.asarray(inputs["mask"], np.int32))
    w_fc = np.asarray(inputs["w_fc"], np.float32)

    if "nc" not in _cache:
        _cache["nc"] = _build_nc()
    nc = _cache["nc"]

    cbs, c2 = _host_blobs(mask, w_fc)
    in_maps = [
        {"x": enc[c * BS:(c + 1) * BS], "cb": cbs[c], "c2": c2}
        for c in range(NCORES)
    ]
    res = bass_utils.run_bass_kernel_spmd(
        nc, in_maps, core_ids=list(range(NCORES))
    )
    o = np.stack([r["out"] for r in res.results]).astype(np.float64)
    num = o[:, :, 0].sum()
    den = o[:, :, 1].sum() - B
    return np.asarray(num / den, dtype=np.float32)


# revision 12
# speedup vs baseline: 1.0457x; 1.0457x over previous
"""Trainium2 Bass kernel for nn_DLI_loss_full.

Key algebraic fact: logits[b,j,k] = hw[b,j] + xw[b,k] and the loss is
sum(lse - tgt) over valid groups, so the hw[b,j] term (the whole LSTM
path) cancels exactly:

    per_group[b,j] = log(sum_{k=j+1}^{len_b-1} exp(xw[b,k])) - xw[b,j+1]
    loss = sum(per_group) / sum_b(len_b - 1)

with xw = encoder_output @ w_fc[HID:].  The kernel streams
encoder_output once (memory-bound, ~351 B/ns on the scalar HWDGE queue
with all 16 DMA engines saturated; nothing else rides that queue) and
computes the per-timestep dot products with all three compute engines
balanced against the stream:

  - scalar (ACT) casts each landed fp32 piece to bf16, and one piece's
    256-wide reductions run there via Copy+accum_out,
  - vector (DVE) multiplies in the 2x 16-bit perf mode; reductions run
    as two bf16 halving adds (2x mode) plus a short 1x reduce,
  - gpsimd multiplies two pieces in fp32 straight from the landed
    tiles (no cast needed).

Constants ride the gpsimd SWDGE queue (independent of the scalar HWDGE
stream, so they land in the first ~10us); weight replicas are built
on-chip while the first piece is in flight.  A manual LoadActFuncSet
of natural_log_exp_and_others at the top of the scalar stream serves
Exp and Ln with a single hidden table load.  The valid-k mask folds
into xw as an additive -60000 bias so a single Exp+accum_out produces
both the masked exponentials and the chunk totals; suffix log-sum-exps
then come from one per-chunk scan seeded through a 128x128 bf16
matmul.
"""

from contextlib import ExitStack

import ml_dtypes
import numpy as np

import concourse.bacc as bacc
import concourse.mybir as mybir
import concourse.tile as tile
from concourse import bass_utils

B, T, D, HID = 128, 384, 256, 256
NCORES = 8
BS = B // NCORES            # 16 batches per core
CH = 8                      # chunks per sequence
L = T // CH                 # 48 timesteps per chunk
P = BS * CH                 # 128 partitions
NP = 8                      # DMA/compute pieces along the free axis
LP = L // NP                # 6 timesteps per piece
F32 = mybir.dt.float32
BF16 = mybir.dt.bfloat16
EPS = 1e-30                 # keeps ln() finite on fully-masked tails
MASK_NEG = -60000.0         # exp() underflows to exactly 0
ATL_LN_EXP = 6              # act_info.json index of natural_log_exp_and_others

GP_MULT = (1, 4)            # pieces whose multiply runs on gpsimd (fp32)
ACT_RED = (2,)              # pieces whose reduce runs on scalar Copy+accum

_cache = {}


def _build_nc():
    nc = bacc.Bacc(
        "TRN2", target_bir_lowering=False, debug=False, num_devices=NCORES
    )
    x = nc.dram_tensor("x", [BS, T, D], F32, kind="ExternalInput").ap()
    cb = nc.dram_tensor("cb", [P, D + 3 * L], F32, kind="ExternalInput").ap()
    c2 = nc.dram_tensor("c2", [P, P], BF16, kind="ExternalInput").ap()
    out = nc.dram_tensor("out", [P, 2], F32, kind="ExternalOutput").ap()

    add = mybir.AluOpType.add
    mult = mybir.AluOpType.mult
    bypass = mybir.AluOpType.bypass
    AX = mybir.AxisListType.X
    ACT = mybir.ActivationFunctionType

    with tile.TileContext(nc) as tc, ExitStack() as ctx, \
            nc.allow_low_precision(reason="bf16 2x-mode dot products; loss tolerance is 2e-2"):
        sp = ctx.enter_context(tc.tile_pool(name="small", bufs=1))
        xp = ctx.enter_context(tc.tile_pool(name="xp", bufs=NP))
        bp = ctx.enter_context(tc.tile_pool(name="bp", bufs=4))
        pp = ctx.enter_context(tc.tile_pool(name="psum", bufs=1, space="PSUM"))

        # one table load serves every Exp/Ln below; runs at stream head
        # while the first x piece is still in flight
        nc.scalar.add_instruction(
            mybir.InstLoadActFuncSet(
                name="manual_atl", act_func_set_id=ATL_LN_EXP, ins=[], outs=[]
            )
        )

        # x-piece loads all on the scalar HWDGE queue: 128 descriptors of
        # 6KB per piece keep all 16 DMA engines saturated (~351 B/ns)
        x_p = x.rearrange("b (c n l) d -> (b c) n (l d)", c=CH, n=NP)
        xts = []
        for i in range(NP):
            xt = xp.tile([P, LP * D], F32, tag="x")
            nc.scalar.dma_start(xt[:], x_p[:, i, :])
            xts.append(xt)

        # constants ride the gpsimd SWDGE queue: independent DMA path,
        # does not steal issue slots from the x stream
        c_sb = sp.tile([P, D + 3 * L], F32)
        nc.gpsimd.dma_start(c_sb[:], cb)
        w_sb = c_sb[:, 0:D]
        mf_sb = c_sb[:, D:D + L]
        wm_sb = c_sb[:, D + L:D + 2 * L]
        mb_sb = c_sb[:, D + 2 * L:D + 3 * L]
        umb = sp.tile([P, P], BF16)
        nc.gpsimd.dma_start(umb[:], c2)

        # weight replicas, built while the first piece is in flight:
        # fp32 x LP on gpsimd (for its own mults), bf16 x LP on vector
        wf = sp.tile([P, LP * D], F32)
        nc.gpsimd.tensor_copy(wf[:, 0:D], w_sb)
        nc.gpsimd.tensor_copy(wf[:, D:2 * D], wf[:, 0:D])
        nc.gpsimd.tensor_copy(wf[:, 2 * D:4 * D], wf[:, 0:2 * D])
        nc.gpsimd.tensor_copy(wf[:, 4 * D:6 * D], wf[:, 2 * D:4 * D])
        w3f = wf[:].rearrange("p (l d) -> p l d", d=D)
        wtb = sp.tile([P, D], BF16)
        nc.scalar.activation(wtb[:], w_sb, ACT.Copy)
        wb = sp.tile([P, LP * D], BF16)
        nc.vector.tensor_copy(wb[:, 0:D], wtb[:])
        nc.vector.tensor_copy(wb[:, D:2 * D], wb[:, 0:D])
        nc.vector.tensor_copy(wb[:, 2 * D:4 * D], wb[:, 0:2 * D])
        nc.vector.tensor_copy(wb[:, 4 * D:6 * D], wb[:, 2 * D:4 * D])
        w3b = wb[:].rearrange("p (l d) -> p l d", d=D)

        res = sp.tile([P, 2], F32)
        # valid-group count is independent of x: do it up front
        nc.vector.tensor_reduce(res[:, 1:2], mf_sb, axis=AX, op=add)

        # xw[p, t] = sum_d x[p, t, d] * w[d]
        xw = sp.tile([P, L], F32)
        ascr = sp.tile([P, D], BF16)

        def dve_reduce_staged(x3, i):
            # two bf16 halving adds in the 2x mode, then a short 1x reduce
            nc.vector.tensor_tensor(
                x3[:, :, 0:128], x3[:, :, 0:128], x3[:, :, 128:256], add
            )
            nc.vector.tensor_tensor(
                x3[:, :, 0:64], x3[:, :, 0:64], x3[:, :, 64:128], add
            )
            nc.vector.tensor_reduce(
                xw[:, i * LP:(i + 1) * LP], x3[:, :, 0:64], axis=AX, op=add
            )

        for i in range(NP):
            if i in GP_MULT:
                x3 = xts[i][:].rearrange("p (l d) -> p l d", d=D)
                nc.gpsimd.tensor_tensor(x3, x3, w3f, mult)
                nc.vector.tensor_reduce(
                    xw[:, i * LP:(i + 1) * LP], x3, axis=AX, op=add
                )
            elif i < NP - 1:
                xb = bp.tile([P, LP * D], BF16, tag="xb")
                nc.scalar.activation(xb[:], xts[i][:], ACT.Copy)
                x3 = xb[:].rearrange("p (l d) -> p l d", d=D)
                nc.vector.tensor_tensor(x3, x3, w3b, mult)
                if i in ACT_RED:
                    for l in range(LP):
                        t = i * LP + l
                        nc.scalar.activation(
                            ascr[:], xb[:, l * D:(l + 1) * D], ACT.Copy,
                            accum_out=xw[:, t:t + 1],
                        )
                else:
                    dve_reduce_staged(x3, i)
            else:
                # last piece in three 2-row chips to shorten the serial
                # cast->mult->reduce tail after its DMA lands
                xb = bp.tile([P, LP * D], BF16, tag="xb")
                x3 = xb[:].rearrange("p (l d) -> p l d", d=D)
                for j in range(3):
                    c0, c1 = j * 2, (j + 1) * 2
                    nc.scalar.activation(
                        xb[:, c0 * D:c1 * D], xts[i][:, c0 * D:c1 * D],
                        ACT.Copy,
                    )
                    nc.vector.tensor_tensor(
                        x3[:, c0:c1], x3[:, c0:c1], w3b[:, c0:c1], mult
                    )
                    nc.vector.tensor_reduce(
                        xw[:, i * LP + c0:i * LP + c1],
                        x3[:, c0:c1], axis=AX, op=add,
                    )

        # fold the valid-k mask into xw (masked -> -60000, exp -> 0);
        # wm is 0 there so the loss terms are unaffected
        nc.vector.tensor_add(xw[:], xw[:], mb_sb)
        # masked exponentials and chunk totals in one activation
        em = sp.tile([P, L], F32)
        tot = sp.tile([P, 1], BF16)
        nc.scalar.activation(em[:], xw[:], ACT.Exp, accum_out=tot[:])
        # cross-chunk exclusive suffix totals via 128x128 bf16 matmul
        aps = pp.tile([P, 1], F32, tag="mm")
        nc.tensor.matmul(aps[:], umb[:], tot[:], start=True, stop=True)
        a_sb = sp.tile([P, 1], F32)
        # + EPS seeds every suffix sum, keeping ln() finite on
        # fully-masked tails
        nc.vector.tensor_scalar_add(a_sb[:], aps[:], EPS)

        # within-chunk suffix sums, seeded with the later-chunk total
        ss = sp.tile([P, L], F32)
        nc.vector.tensor_tensor_scan(
            ss[:][:, ::-1], em[:][:, ::-1], em[:][:, ::-1],
            initial=a_sb[:], op0=add, op1=bypass,
        )
        lt = sp.tile([P, L], F32)
        nc.scalar.activation(lt[:], ss[:], ACT.Ln)

        # loss terms: sum over valid groups of (ln(suffix) - xw)
        diff = sp.tile([P, L], F32)
        nc.vector.tensor_sub(diff[:], lt[:], xw[:])
        nc.vector.scalar_tensor_tensor(
            out=diff[:], in0=diff[:], scalar=1.0, in1=wm_sb,
            op0=bypass, op1=mult, accum_out=res[:, 0:1],
        )
        nc.scalar.dma_start(out, res[:])

    nc.compile()
    return nc


def _host_consts():
    w_idx = np.arange(P)
    um = (
        (w_idx[:, None] // CH == w_idx[None, :] // CH)
        & (w_idx[:, None] % CH > w_idx[None, :] % CH)
    ).astype(np.float32)
    cm = np.ones((P, L), np.float32)
    cm[w_idx % CH == 0, 0] = 0.0
    return um, cm


def _host_blobs(mask, w_fc):
    """Per-core cb fp32 blobs and the shared bf16 um matrix."""
    um, cm = _host_consts()
    wt = np.broadcast_to(w_fc[HID:], (P, D)).astype(np.float32)
    c2 = np.ascontiguousarray(um.astype(ml_dtypes.bfloat16))
    mfs = mask.astype(np.float32).reshape(NCORES, P, L)
    cbs = []
    for c in range(NCORES):
        mf = mfs[c]
        wm = mf * cm
        mb = (1.0 - mf) * MASK_NEG
        cbs.append(np.ascontiguousarray(
            np.concatenate([wt, mf, wm, mb], axis=1), np.float32
        ))
    return cbs, c2


def kernel(**inputs) -> np.ndarray:
    enc = np.ascontiguousarray(np.asarray(inputs["encoder_output"], np.float32))
    mask = np.ascontiguousarray(np.asarray(inputs["mask"], np.int32))
    w_fc = np.asarray(inputs["w_fc"], np.float32)

    if "nc" not in _cache:
        _cache["nc"] = _build_nc()
    nc = _cache["nc"]

    cbs, c2 = _host_blobs(mask, w_fc)
    in_maps = [
        {"x": enc[c * BS:(c + 1) * BS], "cb": cbs[c], "c2": c2}
        for c in range(NCORES)
    ]
    res = bass_utils.run_bass_kernel_spmd(
        nc, in_maps, core_ids=list(range(NCORES))
    )
    o = np.stack([r["out"] for r in res.results]).astype(np.float64)
    num = o[:, :, 0].sum()
    den = o[:, :, 1].sum() - B
    return np.asarray(num / den, dtype=np.float32)


# revision 13
# speedup vs baseline: 1.0701x; 1.0233x over previous
"""Trainium2 Bass kernel for nn_DLI_loss_full.

Key algebraic fact: logits[b,j,k] = hw[b,j] + xw[b,k] and the loss is
sum(lse - tgt) over valid groups, so the hw[b,j] term (the whole LSTM
path) cancels exactly:

    per_group[b,j] = log(sum_{k=j+1}^{len_b-1} exp(xw[b,k])) - xw[b,j+1]
    loss = sum(per_group) / sum_b(len_b - 1)

with xw = encoder_output @ w_fc[HID:].  The kernel streams
encoder_output once (memory-bound, ~351 B/ns on the scalar HWDGE queue
with all 16 DMA engines saturated; w rides the same queue first so the
weight replicas exist before the first piece lands) and computes the
per-timestep dot products with the engines balanced against the
stream:

  - scalar (ACT) casts each landed fp32 piece to bf16,
  - vector (DVE) multiplies in the 2x 16-bit perf mode and reduces via
    two bf16 halving adds (2x mode) plus a short 1x reduce,
  - gpsimd multiplies two pieces in fp32 straight from the landed
    tiles (no cast needed; vector picks up their reduces mid-stream).

tile_wait_until hints pin the scheduler's per-engine instruction order
to the real DMA landing cadence — without them the compile-time
schedule head-of-line-blocks the vector engine on gpsimd results.
Small constants (masks, um) ride the gpsimd SWDGE queue; they are only
needed in the tail.  A manual LoadActFuncSet of
natural_log_exp_and_others at the top of the scalar stream serves Exp
and Ln with a single hidden table load.  The valid-k mask folds into
xw as an additive -60000 bias so a single Exp+accum_out produces both
the masked exponentials and the chunk totals; suffix log-sum-exps come
from one per-chunk scan seeded through a 128x128 bf16 matmul.
"""

from contextlib import ExitStack

import ml_dtypes
import numpy as np

import concourse.bacc as bacc
import concourse.mybir as mybir
import concourse.tile as tile
from concourse import bass_utils

B, T, D, HID = 128, 384, 256, 256
NCORES = 8
BS = B // NCORES            # 16 batches per core
CH = 8                      # chunks per sequence
L = T // CH                 # 48 timesteps per chunk
P = BS * CH                 # 128 partitions
NP = 8                      # DMA/compute pieces along the free axis
LP = L // NP                # 6 timesteps per piece
F32 = mybir.dt.float32
BF16 = mybir.dt.bfloat16
EPS = 1e-30                 # keeps ln() finite on fully-masked tails
MASK_NEG = -60000.0         # exp() underflows to exactly 0
ATL_LN_EXP = 6              # act_info.json index of natural_log_exp_and_others

GP_MULT = (1, 4)            # pieces whose multiply runs on gpsimd (fp32)

_cache = {}


def _build_nc():
    nc = bacc.Bacc(
        "TRN2", target_bir_lowering=False, debug=False, num_devices=NCORES
    )
    x = nc.dram_tensor("x", [BS, T, D], F32, kind="ExternalInput").ap()
    wt = nc.dram_tensor("wt", [P, D], F32, kind="ExternalInput").ap()
    cb = nc.dram_tensor("cb", [P, 3 * L], F32, kind="ExternalInput").ap()
    c2 = nc.dram_tensor("c2", [P, P], BF16, kind="ExternalInput").ap()
    out = nc.dram_tensor("out", [P, 2], F32, kind="ExternalOutput").ap()

    add = mybir.AluOpType.add
    mult = mybir.AluOpType.mult
    bypass = mybir.AluOpType.bypass
    AX = mybir.AxisListType.X
    ACT = mybir.ActivationFunctionType

    with tile.TileContext(nc) as tc, ExitStack() as ctx, \
            nc.allow_low_precision(reason="bf16 2x-mode dot products; loss tolerance is 2e-2"):
        sp = ctx.enter_context(tc.tile_pool(name="small", bufs=1))
        xp = ctx.enter_context(tc.tile_pool(name="xp", bufs=NP))
        bp = ctx.enter_context(tc.tile_pool(name="bp", bufs=4))
        pp = ctx.enter_context(tc.tile_pool(name="psum", bufs=1, space="PSUM"))

        # expected piece-i landing time in the scheduler's simulated
        # clock (serial DMA queue model): used only to pin instruction
        # order per engine, no hardware waits are emitted
        def tw(us):
            return tc.tile_wait_until(us * 1e-3)

        t_piece = [0.8 + 2.37 * (i + 1) for i in range(NP)]

        # one table load serves every Exp/Ln below; runs at stream head
        # while w / the first x piece are still in flight
        nc.scalar.add_instruction(
            mybir.InstLoadActFuncSet(
                name="manual_atl", act_func_set_id=ATL_LN_EXP, ins=[], outs=[]
            )
        )

        # w first, then the x pieces, all on the scalar HWDGE queue: 128
        # descriptors of 6KB per piece keep all 16 DMA engines saturated
        w_sb = sp.tile([P, D], F32)
        nc.scalar.dma_start(w_sb[:], wt)
        x_p = x.rearrange("b (c n l) d -> (b c) n (l d)", c=CH, n=NP)
        xts = []
        for i in range(NP):
            xt = xp.tile([P, LP * D], F32, tag="x")
            nc.scalar.dma_start(xt[:], x_p[:, i, :])
            xts.append(xt)

        # small tail-only constants ride the gpsimd SWDGE queue
        c_sb = sp.tile([P, 3 * L], F32)
        nc.gpsimd.dma_start(c_sb[:], cb)
        mf_sb = c_sb[:, 0:L]
        wm_sb = c_sb[:, L:2 * L]
        mb_sb = c_sb[:, 2 * L:3 * L]
        umb = sp.tile([P, P], BF16)
        nc.gpsimd.dma_start(umb[:], c2)

        # weight replicas, built while the first piece is in flight:
        # fp32 x LP on gpsimd (for its own mults), bf16 x LP on vector
        wf = sp.tile([P, LP * D], F32)
        nc.gpsimd.tensor_copy(wf[:, 0:D], w_sb[:])
        nc.gpsimd.tensor_copy(wf[:, D:2 * D], wf[:, 0:D])
        nc.gpsimd.tensor_copy(wf[:, 2 * D:4 * D], wf[:, 0:2 * D])
        nc.gpsimd.tensor_copy(wf[:, 4 * D:6 * D], wf[:, 2 * D:4 * D])
        w3f = wf[:].rearrange("p (l d) -> p l d", d=D)
        wtb = sp.tile([P, D], BF16)
        nc.scalar.activation(wtb[:], w_sb[:], ACT.Copy)
        wb = sp.tile([P, LP * D], BF16)
        nc.vector.tensor_copy(wb[:, 0:D], wtb[:])
        nc.vector.tensor_copy(wb[:, D:2 * D], wb[:, 0:D])
        nc.vector.tensor_copy(wb[:, 2 * D:4 * D], wb[:, 0:2 * D])
        nc.vector.tensor_copy(wb[:, 4 * D:6 * D], wb[:, 2 * D:4 * D])
        w3b = wb[:].rearrange("p (l d) -> p l d", d=D)

        res = sp.tile([P, 2], F32)
        # valid-group count is independent of x: do it up front
        nc.vector.tensor_reduce(res[:, 1:2], mf_sb, axis=AX, op=add)

        # xw[p, t] = sum_d x[p, t, d] * w[d]
        xw = sp.tile([P, L], F32)

        def dve_reduce_staged(x3, i):
            # two bf16 halving adds in the 2x mode, then a short 1x reduce
            nc.vector.tensor_tensor(
                x3[:, :, 0:128], x3[:, :, 0:128], x3[:, :, 128:256], add
            )
            nc.vector.tensor_tensor(
                x3[:, :, 0:64], x3[:, :, 0:64], x3[:, :, 64:128], add
            )
            nc.vector.tensor_reduce(
                xw[:, i * LP:(i + 1) * LP], x3[:, :, 0:64], axis=AX, op=add
            )

        for i in range(NP):
            if i in GP_MULT:
                x3 = xts[i][:].rearrange("p (l d) -> p l d", d=D)
                with tw(t_piece[i]):
                    nc.gpsimd.tensor_tensor(x3, x3, w3f, mult)
                with tw(t_piece[i] + 3.5):
                    nc.vector.tensor_reduce(
                        xw[:, i * LP:(i + 1) * LP], x3, axis=AX, op=add
                    )
            elif i < NP - 1:
                xb = bp.tile([P, LP * D], BF16, tag="xb")
                x3 = xb[:].rearrange("p (l d) -> p l d", d=D)
                with tw(t_piece[i]):
                    nc.scalar.activation(xb[:], xts[i][:], ACT.Copy)
                with tw(t_piece[i] + 1.3):
                    nc.vector.tensor_tensor(x3, x3, w3b, mult)
                with tw(t_piece[i] + 2.2):
                    dve_reduce_staged(x3, i)
            else:
                # last piece in three 2-row chips to shorten the serial
                # cast->mult->reduce tail after its DMA lands
                xb = bp.tile([P, LP * D], BF16, tag="xb")
                x3 = xb[:].rearrange("p (l d) -> p l d", d=D)
                for j in range(3):
                    c0, c1 = j * 2, (j + 1) * 2
                    with tw(t_piece[i] + 0.45 * j):
                        nc.scalar.activation(
                            xb[:, c0 * D:c1 * D], xts[i][:, c0 * D:c1 * D],
                            ACT.Copy,
                        )
                    with tw(t_piece[i] + 0.45 * j + 0.5):
                        nc.vector.tensor_tensor(
                            x3[:, c0:c1], x3[:, c0:c1], w3b[:, c0:c1], mult
                        )
                        nc.vector.tensor_reduce(
                            xw[:, i * LP + c0:i * LP + c1],
                            x3[:, c0:c1], axis=AX, op=add,
                        )

        TT = t_piece[NP - 1] + 2.0
        # fold the valid-k mask into xw (masked -> -60000, exp -> 0);
        # wm is 0 there so the loss terms are unaffected
        with tw(TT):
            nc.vector.tensor_add(xw[:], xw[:], mb_sb)
        # masked exponentials and chunk totals in one activation
        em = sp.tile([P, L], F32)
        tot = sp.tile([P, 1], BF16)
        with tw(TT + 0.1):
            nc.scalar.activation(em[:], xw[:], ACT.Exp, accum_out=tot[:])
        # cross-chunk exclusive suffix totals via 128x128 bf16 matmul
        aps = pp.tile([P, 1], F32, tag="mm")
        with tw(TT + 0.2):
            nc.tensor.matmul(aps[:], umb[:], tot[:], start=True, stop=True)
        a_sb = sp.tile([P, 1], F32)
        # + EPS seeds every suffix sum, keeping ln() finite on
        # fully-masked tails
        with tw(TT + 0.3):
            nc.vector.tensor_scalar_add(a_sb[:], aps[:], EPS)

        # within-chunk suffix sums, seeded with the later-chunk total
        ss = sp.tile([P, L], F32)
        with tw(TT + 0.4):
            nc.vector.tensor_tensor_scan(
                ss[:][:, ::-1], em[:][:, ::-1], em[:][:, ::-1],
                initial=a_sb[:], op0=add, op1=bypass,
            )
        lt = sp.tile([P, L], F32)
        with tw(TT + 0.5):
            nc.scalar.activation(lt[:], ss[:], ACT.Ln)

        # loss terms: sum over valid groups of (ln(suffix) - xw)
        diff = sp.tile([P, L], F32)
        with tw(TT + 0.6):
            nc.vector.tensor_sub(diff[:], lt[:], xw[:])
            nc.vector.scalar_tensor_tensor(
                out=diff[:], in0=diff[:], scalar=1.0, in1=wm_sb,
                op0=bypass, op1=mult, accum_out=res[:, 0:1],
            )
        with tw(TT + 0.8):
            nc.scalar.dma_start(out, res[:])

    nc.compile()
    return nc


def _host_consts():
    w_idx = np.arange(P)
    um = (
        (w_idx[:, None] // CH == w_idx[None, :] // CH)
        & (w_idx[:, None] % CH > w_idx[None, :] % CH)
    ).astype(np.float32)
    cm = np.ones((P, L), np.float32)
    cm[w_idx % CH == 0, 0] = 0.0
    return um, cm


def _host_blobs(mask, w_fc):
    """wt fp32, per-core mask blobs fp32, um bf16."""
    um, cm = _host_consts()
    wt = np.ascontiguousarray(
        np.broadcast_to(w_fc[HID:], (P, D)), np.float32
    )
    c2 = np.ascontiguousarray(um.astype(ml_dtypes.bfloat16))
    mfs = mask.astype(np.float32).reshape(NCORES, P, L)
    cbs = []
    for c in range(NCORES):
        mf = mfs[c]
        wm = mf * cm
        mb = (1.0 - mf) * MASK_NEG
        cbs.append(np.ascontiguousarray(
            np.concatenate([mf, wm, mb], axis=1), np.float32
        ))
    return wt, cbs, c2


def kernel(**inputs) -> np.ndarray:
    enc = np.ascontiguousarray(np.asarray(inputs["encoder_output"], np.float32))
    mask = np.ascontiguousarray(np.asarray(inputs["mask"], np.int32))
    w_fc = np.asarray(inputs["w_fc"], np.float32)

    if "nc" not in _cache:
        _cache["nc"] = _build_nc()
    nc = _cache["nc"]

    wt, cbs, c2 = _host_blobs(mask, w_fc)
    in_maps = [
        {"x": enc[c * BS:(c + 1) * BS], "wt": wt, "cb": cbs[c], "c2": c2}
        for c in range(NCORES)
    ]
    res = bass_utils.run_bass_kernel_spmd(
        nc, in_maps, core_ids=list(range(NCORES))
    )
    o = np.stack([r["out"] for r in res.results]).astype(np.float64)
    num = o[:, :, 0].sum()
    den = o[:, :, 1].sum() - B
    return np.asarray(num / den, dtype=np.float32)


# revision 18
# speedup vs baseline: 1.2269x; 1.1466x over previous
"""Trainium2 Bass kernel for nn_DLI_loss_full.

Key algebraic fact: logits[b,j,k] = hw[b,j] + xw[b,k] and the loss is
sum(lse - tgt) over valid groups, so the hw[b,j] term (the whole LSTM
path) cancels exactly:

    per_group[b,j] = log(sum_{k=j+1}^{len_b-1} exp(xw[b,k])) - xw[b,j+1]
    loss = sum(per_group) / sum_b(len_b - 1)

with xw = encoder_output @ w_fc[HID:].  The kernel streams
encoder_output once (memory-bound, ~351 B/ns with all 16 DMA engines
saturated) on the sync HWDGE queue — the sync sequencer has nothing
else to do, so descriptor generation never delays compute issue — and
computes the per-timestep dot products as a two-engine pipeline with
no cross-engine back-edges (so the compile-time schedule cannot
head-of-line-block an engine):

  - scalar (ACT) casts each landed fp32 piece to bf16,
  - vector (DVE) multiplies in the 2x 16-bit perf mode and reduces via
    two bf16 halving adds (2x mode) plus a short 1x reduce.

tile_wait_until hints align the scheduler's per-engine instruction
order with the real DMA landing cadence.
Small constants (masks, um) ride the gpsimd SWDGE queue; they are only
needed in the tail.  A manual LoadActFuncSet of
natural_log_exp_and_others at the top of the scalar stream serves Exp
and Ln with a single hidden table load.  The valid-k mask folds into
xw as an additive -60000 bias so a single Exp+accum_out produces both
the masked exponentials and the chunk totals; suffix log-sum-exps come
from one per-chunk scan seeded through a 128x128 bf16 matmul.
"""

from contextlib import ExitStack

import ml_dtypes
import numpy as np

import concourse.bacc as bacc
import concourse.mybir as mybir
import concourse.tile as tile
from concourse import bass_utils

B, T, D, HID = 128, 384, 256, 256
NCORES = 8
BS = B // NCORES            # 16 batches per core
CH = 8                      # chunks per sequence
L = T // CH                 # 48 timesteps per chunk
P = BS * CH                 # 128 partitions
NP = 8                      # DMA/compute pieces along the free axis
LP = L // NP                # 6 timesteps per piece
F32 = mybir.dt.float32
BF16 = mybir.dt.bfloat16
EPS = 1e-30                 # keeps ln() finite on fully-masked tails
MASK_NEG = -60000.0         # exp() underflows to exactly 0
ATL_LN_EXP = 6              # act_info.json index of natural_log_exp_and_others

_cache = {}


def _build_nc():
    nc = bacc.Bacc(
        "TRN2", target_bir_lowering=False, debug=False, num_devices=NCORES
    )
    x = nc.dram_tensor("x", [BS, T, D], F32, kind="ExternalInput").ap()
    wt = nc.dram_tensor("wt", [P, D], F32, kind="ExternalInput").ap()
    cb = nc.dram_tensor("cb", [P, 3 * L], F32, kind="ExternalInput").ap()
    c2 = nc.dram_tensor("c2", [P, P], BF16, kind="ExternalInput").ap()
    out = nc.dram_tensor("out", [P, 2], F32, kind="ExternalOutput").ap()

    add = mybir.AluOpType.add
    mult = mybir.AluOpType.mult
    bypass = mybir.AluOpType.bypass
    AX = mybir.AxisListType.X
    ACT = mybir.ActivationFunctionType

    with tile.TileContext(nc) as tc, ExitStack() as ctx, \
            nc.allow_low_precision(reason="bf16 2x-mode dot products; loss tolerance is 2e-2"):
        sp = ctx.enter_context(tc.tile_pool(name="small", bufs=1))
        xp = ctx.enter_context(tc.tile_pool(name="xp", bufs=NP))
        bp = ctx.enter_context(tc.tile_pool(name="bp", bufs=4))
        pp = ctx.enter_context(tc.tile_pool(name="psum", bufs=1, space="PSUM"))

        # expected piece-i landing time in the scheduler's simulated
        # clock (serial DMA queue model): used only to pin instruction
        # order per engine, no hardware waits are emitted
        def tw(us):
            return tc.tile_wait_until(us * 1e-3)

        t_piece = [0.8 + 2.37 * (i + 1) for i in range(NP)]

        # one table load serves every Exp/Ln below; runs at stream head
        # while w / the first x piece are still in flight
        nc.scalar.add_instruction(
            mybir.InstLoadActFuncSet(
                name="manual_atl", act_func_set_id=ATL_LN_EXP, ins=[], outs=[]
            )
        )

        # w first, then the x pieces, all on the sync HWDGE queue: 128
        # descriptors of 6KB per piece keep all 16 DMA engines saturated,
        # and the sync sequencer has nothing else to do, so descriptor
        # generation never blocks compute issue (on the scalar queue the
        # 9 descriptor-gens delayed every activation by ~7us)
        w_sb = sp.tile([P, D], F32)
        nc.sync.dma_start(w_sb[:], wt)
        x_p = x.rearrange("b (c n l) d -> (b c) n (l d)", c=CH, n=NP)
        xts = []
        for i in range(NP):
            xt = xp.tile([P, LP * D], F32, tag="x")
            nc.sync.dma_start(xt[:], x_p[:, i, :])
            xts.append(xt)

        # small tail-only constants ride the gpsimd SWDGE queue
        c_sb = sp.tile([P, 3 * L], F32)
        nc.gpsimd.dma_start(c_sb[:], cb)
        mf_sb = c_sb[:, 0:L]
        wm_sb = c_sb[:, L:2 * L]
        mb_sb = c_sb[:, 2 * L:3 * L]
        umb = sp.tile([P, P], BF16)
        nc.gpsimd.dma_start(umb[:], c2)

        # bf16 weight replica, built while the first piece is in flight
        wtb = sp.tile([P, D], BF16)
        nc.scalar.activation(wtb[:], w_sb[:], ACT.Copy)
        wb = sp.tile([P, LP * D], BF16)
        nc.vector.tensor_copy(wb[:, 0:D], wtb[:])
        nc.vector.tensor_copy(wb[:, D:2 * D], wb[:, 0:D])
        nc.vector.tensor_copy(wb[:, 2 * D:4 * D], wb[:, 0:2 * D])
        nc.vector.tensor_copy(wb[:, 4 * D:6 * D], wb[:, 2 * D:4 * D])
        w3b = wb[:].rearrange("p (l d) -> p l d", d=D)

        res = sp.tile([P, 2], F32)
        # valid-group count is independent of x: do it up front
        nc.vector.tensor_reduce(res[:, 1:2], mf_sb, axis=AX, op=add)

        # xw[p, t] = sum_d x[p, t, d] * w[d]
        xw = sp.tile([P, L], F32)

        def dve_reduce_staged(x3, i):
            # two bf16 halving adds in the 2x mode, then a short 1x reduce
            nc.vector.tensor_tensor(
                x3[:, :, 0:128], x3[:, :, 0:128], x3[:, :, 128:256], add
            )
            nc.vector.tensor_tensor(
                x3[:, :, 0:64], x3[:, :, 0:64], x3[:, :, 64:128], add
            )
            nc.vector.tensor_reduce(
                xw[:, i * LP:(i + 1) * LP], x3[:, :, 0:64], axis=AX, op=add
            )

        for i in range(NP):
            if i < NP - 1:
                xb = bp.tile([P, LP * D], BF16, tag="xb")
                x3 = xb[:].rearrange("p (l d) -> p l d", d=D)
                with tw(t_piece[i]):
                    nc.scalar.activation(xb[:], xts[i][:], ACT.Copy)
                with tw(t_piece[i] + 1.3):
                    nc.vector.tensor_tensor(x3, x3, w3b, mult)
                with tw(t_piece[i] + 2.2):
                    dve_reduce_staged(x3, i)
            else:
                # last piece in three 2-row chips to shorten the serial
                # cast->mult->reduce tail after its DMA lands
                xb = bp.tile([P, LP * D], BF16, tag="xb")
                x3 = xb[:].rearrange("p (l d) -> p l d", d=D)
                for j in range(3):
                    c0, c1 = j * 2, (j + 1) * 2
                    with tw(t_piece[i] + 0.45 * j):
                        nc.scalar.activation(
                            xb[:, c0 * D:c1 * D], xts[i][:, c0 * D:c1 * D],
                            ACT.Copy,
                        )
                    with tw(t_piece[i] + 0.45 * j + 0.5):
                        nc.vector.tensor_tensor(
                            x3[:, c0:c1], x3[:, c0:c1], w3b[:, c0:c1], mult
                        )
                        nc.vector.tensor_reduce(
                            xw[:, i * LP + c0:i * LP + c1],
                            x3[:, c0:c1], axis=AX, op=add,
                        )

        TT = t_piece[NP - 1] + 2.0
        # fold the valid-k mask into xw (masked -> -60000, exp -> 0);
        # wm is 0 there so the loss terms are unaffected
        with tw(TT):
            nc.vector.tensor_add(xw[:], xw[:], mb_sb)
        # masked exponentials and chunk totals in one activation
        em = sp.tile([P, L], F32)
        tot = sp.tile([P, 1], BF16)
        with tw(TT + 0.1):
            nc.scalar.activation(em[:], xw[:], ACT.Exp, accum_out=tot[:])
        # cross-chunk exclusive suffix totals via 128x128 bf16 matmul
        aps = pp.tile([P, 1], F32, tag="mm")
        with tw(TT + 0.2):
            nc.tensor.matmul(aps[:], umb[:], tot[:], start=True, stop=True)
        a_sb = sp.tile([P, 1], F32)
        # + EPS seeds every suffix sum, keeping ln() finite on
        # fully-masked tails
        with tw(TT + 0.3):
            nc.vector.tensor_scalar_add(a_sb[:], aps[:], EPS)

        # within-chunk suffix sums, seeded with the later-chunk total
        ss = sp.tile([P, L], F32)
        with tw(TT + 0.4):
            nc.vector.tensor_tensor_scan(
                ss[:][:, ::-1], em[:][:, ::-1], em[:][:, ::-1],
                initial=a_sb[:], op0=add, op1=bypass,
            )
        lt = sp.tile([P, L], F32)
        with tw(TT + 0.5):
            nc.scalar.activation(lt[:], ss[:], ACT.Ln)

        # loss terms: sum over valid groups of (ln(suffix) - xw)
        diff = sp.tile([P, L], F32)
        with tw(TT + 0.6):
            nc.vector.tensor_sub(diff[:], lt[:], xw[:])
            nc.vector.scalar_tensor_tensor(
                out=diff[:], in0=diff[:], scalar=1.0, in1=wm_sb,
                op0=bypass, op1=mult, accum_out=res[:, 0:1],
            )
        with tw(TT + 0.8):
            nc.scalar.dma_start(out, res[:])

    nc.compile()
    return nc


def _host_consts():
    w_idx = np.arange(P)
    um = (
        (w_idx[:, None] // CH == w_idx[None, :] // CH)
        & (w_idx[:, None] % CH > w_idx[None, :] % CH)
    ).astype(np.float32)
    cm = np.ones((P, L), np.float32)
    cm[w_idx % CH == 0, 0] = 0.0
    return um, cm


def _host_blobs(mask, w_fc):
    """wt fp32, per-core mask blobs fp32, um bf16."""
    um, cm = _host_consts()
    wt = np.ascontiguousarray(
        np.broadcast_to(w_fc[HID:], (P, D)), np.float32
    )
    c2 = np.ascontiguousarray(um.astype(ml_dtypes.bfloat16))
    mfs = mask.astype(np.float32).reshape(NCORES, P, L)
    cbs = []
    for c in range(NCORES):
        mf = mfs[c]
        wm = mf * cm
        mb = (1.0 - mf) * MASK_NEG
        cbs.append(np.ascontiguousarray(
            np.concatenate([mf, wm, mb], axis=1), np.float32
        ))
    return wt, cbs, c2


def kernel(**inputs) -> np.ndarray:
    enc = np.ascontiguousarray(np.asarray(inputs["encoder_output"], np.float32))
    mask = np.ascontiguousarray(np.asarray(inputs["mask"], np.int32))
    w_fc = np.asarray(inputs["w_fc"], np.float32)

    if "nc" not in _cache:
        _cache["nc"] = _build_nc()
    nc = _cache["nc"]

    wt, cbs, c2 = _host_blobs(mask, w_fc)
    in_maps = [
        {"x": enc[c * BS:(c + 1) * BS], "wt": wt, "cb": cbs[c], "c2": c2}
        for c in range(NCORES)
    ]
    res = bass_utils.run_bass_kernel_spmd(
        nc, in_maps, core_ids=list(range(NCORES))
    )
    o = np.stack([r["out"] for r in res.results]).astype(np.float64)
    num = o[:, :, 0].sum()
    den = o[:, :, 1].sum() - B
    return np.asarray(num / den, dtype=np.float32)
